# revision 23
# baseline (speedup 1.0000x reference)
"""Trainium2 Bass kernel for nn_BrainInspiredRouter.

Math (reference, seq_len==1 attention => attn collapses to the V path):
    attended = x @ (out_proj_w @ Wv).T + (out_proj_w @ bv + out_proj_b)
    h        = relu(attended @ W1[r].T + b1[r])          per route r
    route    = h @ W2[r].T + b2[r]
    gate     = softmax(x @ Wg.T + bg)
    out      = sum_r gate[:, r] * route[:, r, :]

Host-side constant folding (weights only, no activations):
    W1f[r]  = W1[r] @ (out_proj_w @ Wv)      -> h = relu(x @ W1f.T + b1f)
    b1f[r]  = W1[r] @ (out_proj_w@bv + out_proj_b) + b1[r]
    W2cat   = W2.transpose(0,2,1).reshape(R*DH, DOUT)
    out     = (gate*h_flat) @ W2cat + gate @ b2

Device (per core, batch-sharded 8 ways, 2048 rows each; feature-major "T"
layout so both GEMMs chain without transposes):
    gate per chunk: logitsT[8,b] -> E=exp(+bg) -> gate_bf (bf16, unnorm);
      per-route partition-broadcast of E rows and of 1/S via K=1 matmuls
      (ones[1,128].T @ row[1,b]) evicted with ACT Relu (exact for E,1/S>0)
      -- no DRAM round-trip, so the chain has ~1us latency, not ~14us.
    main loop per 512-col batch chunk:
      GEMM1: psum[h,b] = sum_k w1T[k,h-tile] x xT[k,b]   (bf16 MMs)
      evict: ACT relu(+b1f) -> f32 tmp; DVE tmp*gate_bcast -> bf16 Hg
      GEMM2: psum[o,b] = sum_k2 w2[k2,o-tile] x Hg[k2,b] + b2 x gate_bf
      evict: DVE ps2*(1/S) -> f32 -> DMA outT

DMA design: HW queue FIFO order == doorbell order == per-engine program
order; every transfer stripes across all 16 DMA engines; doorbell
instructions cost ~600ns each and block on ring-full (data-paced).  So the
sync stream carries ONLY the bulk loads in need-order (x0 per-k so gate
logits pipeline with arrival, w1 in 8 contiguous 1MB waves with x1 rung
mid-stream), while tiny consts ride the scalar-engine doorbell stream.
"""

import numpy as np

B, D, DOUT, R = 16384, 1024, 1024, 8
DH = D // 2            # 512
RH = R * DH            # 4096
NCORES = 8
BS = B // NCORES       # 2048 rows per core
CHUNK = 512
NCHUNK = BS // CHUNK   # 4
KT = D // 128          # 8 k-tiles over D
HT = RH // 128         # 32 h-tiles
K2T = RH // 128        # 32 k-tiles over RH
OT = DOUT // 128       # 8 out-tiles
GRP = DH // 128        # 4 h-tiles per route
NWAVE = 8              # w1 arrival waves
WHT = HT // NWAVE      # 4 h-tiles per wave

_NC_CACHE = {}


def _build_nc(mm_dt_name="bfloat16"):
    from contextlib import ExitStack

    import concourse.bass as bass
    import concourse.mybir as mybir
    import concourse.tile as tile
    from concourse import bacc

    mm_dt = getattr(mybir.dt, mm_dt_name)
    f32 = mybir.dt.float32
    AF = mybir.ActivationFunctionType

    nc = bacc.Bacc("TRN2", target_bir_lowering=False, debug=False,
                   num_devices=NCORES)

    # host-packed so every bulk load is ONE contiguous-source doorbell
    xw = nc.dram_tensor("xw", [NCHUNK, 128, KT, CHUNK], mm_dt,
                        kind="ExternalInput")
    w1w = nc.dram_tensor("w1w", [NWAVE, 128, KT, WHT * 128], mm_dt,
                         kind="ExternalInput")
    b1v = nc.dram_tensor("b1v", [128, HT], f32, kind="ExternalInput")
    w2 = nc.dram_tensor("w2", [OT, 128, RH], mm_dt, kind="ExternalInput")
    b2d = nc.dram_tensor("b2d", [R, DOUT], mm_dt, kind="ExternalInput")
    wgt = nc.dram_tensor("wgt", [128, KT * R], mm_dt, kind="ExternalInput")
    bgd = nc.dram_tensor("bgd", [R, 1], f32, kind="ExternalInput")
    seld = nc.dram_tensor("seld", [R, R * 128], mm_dt, kind="ExternalInput")
    srec_scr = nc.dram_tensor("srec_scr", [1, BS], f32)
    gate_scr = nc.dram_tensor("gate_scr", [R, BS], mm_dt)
    outT = nc.dram_tensor("outT", [NCHUNK, OT, 128, CHUNK], f32,
                          kind="ExternalOutput")

    with tile.TileContext(nc) as tc, ExitStack() as ctx:
        const = ctx.enter_context(tc.tile_pool(name="const", bufs=1))

        # tiny consts ride the scalar doorbell stream; sync is bulk-only
        bg_sb = const.tile([R, 1], f32, tag="bg")
        nc.scalar.dma_start(bg_sb[:], bgd[:, :])
        ones8b = const.tile([R, 1], mm_dt, tag="ones8b")
        nc.any.memset(ones8b[:], 1.0)
        # row-selector lhsT tiles: sel_all[:, r*128:(r+1)*128] has row r all
        # ones -> (sel_r.T @ E)[p, n] = E[r, n], a partition-broadcast with a
        # K=8 base-0 contraction (K=1 at base partition r is illegal).
        sel_all = const.tile([R, R * 128], mm_dt, tag="sel_all")
        nc.scalar.dma_start(sel_all[:], seld[:, :])

        wg_all = const.tile([128, KT * R], mm_dt, tag="wg_all")
        nc.scalar.dma_start(wg_all[:], wgt[:, :])
        wg_sb = [wg_all[:, k * R:(k + 1) * R] for k in range(KT)]
        b1_sb = const.tile([128, HT], f32, tag="b1")
        nc.scalar.dma_start(b1_sb[:], b1v[:, :])
        b2_sb = const.tile([R, DOUT], mm_dt, tag="b2")
        nc.scalar.dma_start(b2_sb[:], b2d[:, :])

        gate_bf = const.tile([R, BS], mm_dt, tag="gatebf")  # exp(logits)

        # resident x and w1: one 3D SBUF tile each, [128, k, cols]
        x_all = const.tile([128, KT, BS], mm_dt, tag="x_all")
        w1_all = const.tile([128, KT, RH], mm_dt, tag="w1_all")

        def x_mm(k, c):
            return x_all[:, k, c * CHUNK:(c + 1) * CHUNK]

        def w1_mm(k, ht):
            return w1_all[:, k, ht * 128:(ht + 1) * 128]

        def emit_x_load(c, per_k=False):
            sl = slice(c * CHUNK, (c + 1) * CHUNK)
            if per_k:
                for k in range(KT):
                    nc.sync.dma_start(x_all[:, k, sl], xw[c, :, k, :])
            else:
                nc.sync.dma_start(x_all[:, :, sl], xw[c, :, :, :])

        def emit_w1_wave(w):
            lo, hi = w * WHT * 128, (w + 1) * WHT * 128
            nc.sync.dma_start(w1_all[:, :, lo:hi], w1w[w, :, :, :])

        gm = ctx.enter_context(tc.tile_pool(name="gm", bufs=2))
        gbcp = ctx.enter_context(tc.tile_pool(name="gbcp", bufs=2))
        srecp = ctx.enter_context(tc.tile_pool(name="srecp", bufs=2))
        hgp = ctx.enter_context(tc.tile_pool(name="hgp", bufs=1))
        tmpp = ctx.enter_context(tc.tile_pool(name="tmpp", bufs=3))
        w2p = ctx.enter_context(tc.tile_pool(name="w2p", bufs=3))
        outp = ctx.enter_context(tc.tile_pool(name="outp", bufs=3))
        p1 = ctx.enter_context(tc.tile_pool(name="p1", bufs=4, space="PSUM"))
        p2 = ctx.enter_context(tc.tile_pool(name="p2", bufs=2, space="PSUM"))
        pbc = ctx.enter_context(tc.tile_pool(name="pbc", bufs=2, space="PSUM"))

        gbcs = {}
        recs = {}
        srecs = {}

        # HAM warm-up: the PE clock sits at 1.2 GHz until ~3.4us of
        # sustained matmul activity.  The first ~12us are DMA-bound idle,
        # so burn them on dummy matmuls over a memset tile -- the real
        # prologue work (gate logits, first h-tiles) then runs at 2.4 GHz.
        warm = const.tile([128, CHUNK], mm_dt, tag="warm")
        nc.any.memset(warm[:], 0.0)

        def emit_ham_warm(n, label):
            for i in range(n):
                pw = pbc.tile([128, CHUNK], f32, tag="pb",
                              name=f"warm_{label}_{i}")
                nc.tensor.matmul(pw[:], warm[:, 0:128], warm[:],
                                 start=True, stop=True)

        def emit_gate_logits(c):
            """gate_bf[:, c] = exp(x@Wg.T + bg) (bf16, unnormalized);
            the 1/sum factor is applied at GEMM2 eviction."""
            sl = slice(c * CHUNK, (c + 1) * CHUNK)
            pg = pbc.tile([R, CHUNK], f32, tag="pb", name=f"pg_{c}")
            for k in range(KT):
                nc.tensor.matmul(pg[:], wg_sb[k][:], x_mm(k, c),
                                 start=(k == 0), stop=(k == KT - 1))
            nc.scalar.activation(gate_bf[:, sl], pg[:], AF.Exp, bias=bg_sb[:])
            gbcs[c] = [None] * R

        def emit_gate_bcast(c, r):
            """E row r -> all 128 partitions via a selector matmul; ACT Relu
            eviction is exact since E > 0.  Used for chunk 0, where the
            ~10us DMA round-trip latency would stall the eviction pipe."""
            sl = slice(c * CHUNK, (c + 1) * CHUNK)
            pb = pbc.tile([128, CHUNK], f32, tag="pb", name=f"pbc{r}_{c}")
            nc.tensor.matmul(pb[:], sel_all[:, r * 128:(r + 1) * 128],
                             gate_bf[:, sl], start=True, stop=True)
            g = gbcp.tile([128, CHUNK], mm_dt, tag=f"gbc{r}",
                          name=f"gbc{r}_{c}")
            nc.scalar.activation(g[:], pb[:], AF.Relu)
            gbcs[c][r] = g

        grows = {}

        def emit_gate_scr_write(c):
            """stage E rows to DRAM for later replicating reads (gpsimd
            stream: idle engine, fires the moment Exp completes)."""
            sl = slice(c * CHUNK, (c + 1) * CHUNK)
            grows[c] = nc.gpsimd.dma_start(gate_scr[:, sl], gate_bf[:, sl])

        def emit_gate_bcast_dma(c, r):
            """E row r -> 128 partitions via replicating DMA from DRAM; free
            of PE cost.  Only for chunks >= 1 (60us+ of lead time)."""
            g = gbcp.tile([128, CHUNK], mm_dt, tag=f"gbc{r}",
                          name=f"gbc{r}_{c}")
            src = bass.AP(gate_scr, r * BS + c * CHUNK, [[0, 128], [1, CHUNK]])
            dma = nc.gpsimd.dma_start(g[:], src)
            tile.add_dep_helper(dma.ins, grows[c].ins,
                                reason="gate bcast read after scr write")
            gbcs[c][r] = g

        def emit_gate_sum(c):
            """S = sum_r E -> 1/S."""
            sl = slice(c * CHUNK, (c + 1) * CHUNK)
            ps = pbc.tile([1, CHUNK], f32, tag="pb", name=f"ps_{c}")
            nc.tensor.matmul(ps[:], ones8b[:], gate_bf[:, sl],
                             start=True, stop=True)
            rec = gm.tile([1, CHUNK], f32, tag="rec", name=f"rec_{c}")
            nc.vector.reciprocal(rec[:], ps[:])
            recs[c] = rec

        def emit_srec_bcast(c):
            """broadcast 1/S to 128 partitions via replicating DMA round-trip
            (an fp32 K=1 matmul would work, but any fp32 matmul in the NEFF
            downclocks the whole warm-state PE 2.4->2.0 GHz).  Triggers ride
            the scalar stream; only 4 of these per kernel, with ~40us of
            lead time before first use."""
            sl = slice(c * CHUNK, (c + 1) * CHUNK)
            w = nc.scalar.dma_start(srec_scr[:, sl], recs[c][:])
            srec = srecp.tile([128, CHUNK], f32, tag="srec",
                              name=f"srec_{c}")
            src = bass.AP(srec_scr, c * CHUNK, [[0, 128], [1, CHUNK]])
            dma = nc.scalar.dma_start(srec[:], src)
            tile.add_dep_helper(dma.ins, w.ins,
                                reason="srec bcast read after scr write")
            srecs[c] = srec

        # --- bulk loads in need-order; queue FIFO preserves it.  wave0
        # leads so GEMM1 can start the moment x0 lands; x0 per-k so the
        # gate logits pipeline with its arrival; x1 rides mid-stream so
        # chunk-1's gate logits never stall the in-order PE.
        emit_x_load(0, per_k=True)
        for w in range(4):
            emit_w1_wave(w)
        emit_x_load(1)
        for w in range(4, NWAVE):
            emit_w1_wave(w)

        emit_ham_warm(18, "a")
        emit_gate_logits(0)
        emit_ham_warm(4, "b")   # bridge the logits -> wave0 idle

        for c in range(NCHUNK):
            sl = slice(c * CHUNK, (c + 1) * CHUNK)
            if c >= 1 and c + 1 < NCHUNK:
                emit_x_load(c + 1)

            hgs = []
            for ht in range(HT):
                # chunk 0's bcasts just-in-time ahead of first use (route r
                # first read at ht=4r; emit it at ht=4r-2)
                if c == 0 and ht % GRP == GRP - 2 and ht // GRP + 1 < R:
                    emit_gate_bcast(0, ht // GRP + 1)
                ps1 = p1.tile([128, CHUNK], f32, tag="ps1")
                for k in range(KT):
                    nc.tensor.matmul(ps1[:], w1_mm(k, ht), x_mm(k, c),
                                     start=(k == 0), stop=(k == KT - 1))
                if c == 0 and ht == 0:
                    # after ht0's MMs so the PE doesn't idle waiting on Exp
                    emit_gate_bcast(0, 0)
                tmp = tmpp.tile([128, CHUNK], f32, tag="tmp",
                                name=f"tmp_{c}_{ht}")
                nc.scalar.activation(tmp[:], ps1[:], AF.Relu,
                                     bias=b1_sb[:, ht:ht + 1])
                hg = hgp.tile([128, CHUNK], mm_dt, tag=f"hg{ht}",
                              name=f"hg{ht}_{c}")
                nc.vector.tensor_mul(hg[:], tmp[:], gbcs[c][ht // GRP][:])
                hgs.append(hg)
                # gate aux scattered through the ht loop: each insertion is
                # one cheap MM + one ACT, with deps long met
                if c == 0:
                    if ht == 9:
                        emit_gate_sum(0)
                    elif ht == 11:
                        emit_srec_bcast(0)
                if c + 1 < NCHUNK:
                    if ht == 8:
                        emit_gate_logits(c + 1)
                        emit_gate_scr_write(c + 1)
                        for r in range(R):
                            emit_gate_bcast_dma(c + 1, r)
                    elif ht == 27:
                        emit_gate_sum(c + 1)
                    elif ht == 29:
                        emit_srec_bcast(c + 1)

            for ot in range(OT):
                w2t = w2p.tile([128, RH], mm_dt, tag="w2t")
                nc.sync.dma_start(w2t[:], w2[ot, :, :])
                ps2 = p2.tile([128, CHUNK], f32, tag="ps2")
                for k2 in range(K2T):
                    nc.tensor.matmul(ps2[:],
                                     w2t[:, k2 * 128:(k2 + 1) * 128],
                                     hgs[k2][:],
                                     start=(k2 == 0), stop=False)
                nc.tensor.matmul(ps2[:], b2_sb[:, ot * 128:(ot + 1) * 128],
                                 gate_bf[:, sl], start=False, stop=True)
                osb = outp.tile([128, CHUNK], f32, tag="osb")
                nc.vector.tensor_mul(osb[:], ps2[:], srecs[c][:])
                nc.sync.dma_start(outT[c, ot, :, :], osb[:])
            del gbcs[c], srecs[c], recs[c]

    nc.compile()
    return nc


def _get_nc(mm_dt_name="bfloat16"):
    if mm_dt_name not in _NC_CACHE:
        _NC_CACHE[mm_dt_name] = _build_nc(mm_dt_name)
    return _NC_CACHE[mm_dt_name]


def _prepare_in_maps(inputs, np_mm_dtype):
    x = np.asarray(inputs["x"], np.float32)
    in_proj_w = np.asarray(inputs["in_proj_w"], np.float32)
    in_proj_b = np.asarray(inputs["in_proj_b"], np.float32)
    out_proj_w = np.asarray(inputs["out_proj_w"], np.float32)
    out_proj_b = np.asarray(inputs["out_proj_b"], np.float32)
    W1 = np.asarray(inputs["W1"], np.float32)
    b1 = np.asarray(inputs["b1"], np.float32)
    W2 = np.asarray(inputs["W2"], np.float32)
    b2 = np.asarray(inputs["b2"], np.float32)
    Wg = np.asarray(inputs["Wg"], np.float32)
    bg = np.asarray(inputs["bg"], np.float32)

    Wv = in_proj_w[2 * D:]
    bv = in_proj_b[2 * D:]
    A = out_proj_w @ Wv                       # [D, D]
    ba = out_proj_w @ bv + out_proj_b         # [D]
    W1r = W1.reshape(RH, D)
    W1f = W1r @ A                             # [RH, D]
    b1f = W1r @ ba + b1.reshape(RH)           # [RH]
    W2cat = W2.transpose(0, 2, 1).reshape(RH, DOUT)

    # w1w[w, p, k, j] = W1f[w*WHT*128 + j, k*128 + p]
    w1w_np = np.ascontiguousarray(
        W1f.reshape(NWAVE, WHT * 128, KT, 128).transpose(0, 3, 2, 1))
    b1v_np = np.ascontiguousarray(b1f.reshape(HT, 128).T)
    w2_np = np.ascontiguousarray(
        W2cat.reshape(K2T, 128, OT, 128).transpose(2, 1, 0, 3)
    ).reshape(OT, 128, RH)
    # [p, k*R+r] = Wg[r, k*128+p]: 128B-contiguous per partition line
    wgt_np = np.ascontiguousarray(Wg.reshape(R, KT, 128).transpose(2, 1, 0)
                                  ).reshape(128, KT * R)
    bg_np = np.ascontiguousarray(bg.reshape(R, 1))

    shared = {
        "w1w": w1w_np.astype(np_mm_dtype),
        "b1v": b1v_np,
        "w2": w2_np.astype(np_mm_dtype),
        "b2d": b2.astype(np_mm_dtype),
        "wgt": wgt_np.astype(np_mm_dtype),
        "bgd": bg_np,
        "seld": np.kron(np.eye(R, dtype=np.float32),
                        np.ones((1, 128), np.float32)).astype(np_mm_dtype),
    }
    in_maps = []
    for cc in range(NCORES):
        xs = x[cc * BS:(cc + 1) * BS]          # [BS, D]
        # xw[c, p, k, j] = x[c*CHUNK + j, k*128 + p]
        xw_np = np.ascontiguousarray(
            xs.reshape(NCHUNK, CHUNK, KT, 128).transpose(0, 3, 2, 1))
        m = dict(shared)
        m["xw"] = xw_np.astype(np_mm_dtype)
        in_maps.append(m)
    return in_maps


def _run(inputs, trace=False, mm_dt_name="bfloat16"):
    import ml_dtypes
    from concourse.bass_utils import run_bass_kernel_spmd

    np_mm = ml_dtypes.bfloat16 if mm_dt_name == "bfloat16" else np.float32
    nc = _get_nc(mm_dt_name)
    in_maps = _prepare_in_maps(inputs, np_mm)
    res = run_bass_kernel_spmd(nc, in_maps, list(range(NCORES)), trace=trace)
    out = np.empty((B, DOUT), np.float32)
    for cc in range(NCORES):
        o = res.results[cc]["outT"]            # [NCHUNK, OT, 128, CHUNK]
        out[cc * BS:(cc + 1) * BS] = (
            o.transpose(0, 3, 1, 2).reshape(BS, DOUT))
    return out, res


def kernel(**inputs):
    out, _ = _run(inputs, trace=False)
    return out


# revision 26
# speedup vs baseline: 1.0094x; 1.0094x over previous
"""Trainium2 Bass kernel for nn_BrainInspiredRouter.

Math (reference, seq_len==1 attention => attn collapses to the V path):
    attended = x @ (out_proj_w @ Wv).T + (out_proj_w @ bv + out_proj_b)
    h        = relu(attended @ W1[r].T + b1[r])          per route r
    route    = h @ W2[r].T + b2[r]
    gate     = softmax(x @ Wg.T + bg)
    out      = sum_r gate[:, r] * route[:, r, :]

Host-side constant folding (weights only, no activations):
    W1f[r]  = W1[r] @ (out_proj_w @ Wv)      -> h = relu(x @ W1f.T + b1f)
    b1f[r]  = W1[r] @ (out_proj_w@bv + out_proj_b) + b1[r]
    W2cat   = W2.transpose(0,2,1).reshape(R*DH, DOUT)
    out     = (gate*h_flat) @ W2cat + gate @ b2

Device (per core, batch-sharded 8 ways, 2048 rows each; feature-major "T"
layout so both GEMMs chain without transposes):
    gate per chunk: logitsT[8,b] -> E=exp(+bg) -> gate_bf (bf16, unnorm);
      per-route partition-broadcast of E rows and of 1/S via K=1 matmuls
      (ones[1,128].T @ row[1,b]) evicted with ACT Relu (exact for E,1/S>0)
      -- no DRAM round-trip, so the chain has ~1us latency, not ~14us.
    main loop per 512-col batch chunk:
      GEMM1: psum[h,b] = sum_k w1T[k,h-tile] x xT[k,b]   (bf16 MMs)
      evict: ACT relu(+b1f) -> f32 tmp; DVE tmp*gate_bcast -> bf16 Hg
      GEMM2: psum[o,b] = sum_k2 w2[k2,o-tile] x Hg[k2,b] + b2 x gate_bf
      evict: DVE ps2*(1/S) -> f32 -> DMA outT

DMA design: HW queue FIFO order == doorbell order == per-engine program
order; every transfer stripes across all 16 DMA engines; doorbell
instructions cost ~600ns each and block on ring-full (data-paced).  So the
sync stream carries ONLY the bulk loads in need-order (x0 per-k so gate
logits pipeline with arrival, w1 in 8 contiguous 1MB waves with x1 rung
mid-stream), while tiny consts ride the scalar-engine doorbell stream.
"""

import numpy as np

B, D, DOUT, R = 16384, 1024, 1024, 8
DH = D // 2            # 512
RH = R * DH            # 4096
NCORES = 8
BS = B // NCORES       # 2048 rows per core
CHUNK = 512
NCHUNK = BS // CHUNK   # 4
KT = D // 128          # 8 k-tiles over D
HT = RH // 128         # 32 h-tiles
K2T = RH // 128        # 32 k-tiles over RH
OT = DOUT // 128       # 8 out-tiles
GRP = DH // 128        # 4 h-tiles per route
NWAVE = 8              # w1 arrival waves
WHT = HT // NWAVE      # 4 h-tiles per wave

_NC_CACHE = {}


def _build_nc(mm_dt_name="bfloat16"):
    from contextlib import ExitStack

    import concourse.bass as bass
    import concourse.mybir as mybir
    import concourse.tile as tile
    from concourse import bacc

    mm_dt = getattr(mybir.dt, mm_dt_name)
    f32 = mybir.dt.float32
    AF = mybir.ActivationFunctionType

    nc = bacc.Bacc("TRN2", target_bir_lowering=False, debug=False,
                   num_devices=NCORES)

    # host-packed so every bulk load is ONE contiguous-source doorbell
    xw = nc.dram_tensor("xw", [NCHUNK, 128, KT, CHUNK], mm_dt,
                        kind="ExternalInput")
    # chunk 0 again, laid out k-major so each per-k transfer is one
    # contiguous 128KB block (the k-sliced xw view is 1KB-strided and ~3x
    # slower to deliver, starving the gate logits and first h-tiles)
    x0k = nc.dram_tensor("x0k", [KT, 128, CHUNK], mm_dt,
                         kind="ExternalInput")
    w1w = nc.dram_tensor("w1w", [NWAVE, 128, KT, WHT * 128], mm_dt,
                         kind="ExternalInput")
    b1v = nc.dram_tensor("b1v", [128, HT], f32, kind="ExternalInput")
    w2 = nc.dram_tensor("w2", [OT, 128, RH], mm_dt, kind="ExternalInput")
    b2d = nc.dram_tensor("b2d", [R, DOUT], mm_dt, kind="ExternalInput")
    wgt = nc.dram_tensor("wgt", [128, KT * R], mm_dt, kind="ExternalInput")
    bgd = nc.dram_tensor("bgd", [R, 1], f32, kind="ExternalInput")
    seld = nc.dram_tensor("seld", [R, R * 128], mm_dt, kind="ExternalInput")
    srec_scr = nc.dram_tensor("srec_scr", [1, BS], f32)
    gate_scr = nc.dram_tensor("gate_scr", [R, BS], mm_dt)
    outT = nc.dram_tensor("outT", [NCHUNK, OT, 128, CHUNK], f32,
                          kind="ExternalOutput")

    with tile.TileContext(nc) as tc, ExitStack() as ctx:
        const = ctx.enter_context(tc.tile_pool(name="const", bufs=1))

        # tiny consts ride the scalar doorbell stream; sync is bulk-only
        bg_sb = const.tile([R, 1], f32, tag="bg")
        nc.scalar.dma_start(bg_sb[:], bgd[:, :])
        ones8b = const.tile([R, 1], mm_dt, tag="ones8b")
        nc.any.memset(ones8b[:], 1.0)
        # row-selector lhsT tiles: sel_all[:, r*128:(r+1)*128] has row r all
        # ones -> (sel_r.T @ E)[p, n] = E[r, n], a partition-broadcast with a
        # K=8 base-0 contraction (K=1 at base partition r is illegal).
        sel_all = const.tile([R, R * 128], mm_dt, tag="sel_all")
        nc.scalar.dma_start(sel_all[:], seld[:, :])

        wg_all = const.tile([128, KT * R], mm_dt, tag="wg_all")
        nc.scalar.dma_start(wg_all[:], wgt[:, :])
        wg_sb = [wg_all[:, k * R:(k + 1) * R] for k in range(KT)]
        b1_sb = const.tile([128, HT], f32, tag="b1")
        nc.scalar.dma_start(b1_sb[:], b1v[:, :])
        b2_sb = const.tile([R, DOUT], mm_dt, tag="b2")
        nc.scalar.dma_start(b2_sb[:], b2d[:, :])

        gate_bf = const.tile([R, BS], mm_dt, tag="gatebf")  # exp(logits)

        # resident x and w1: one 3D SBUF tile each, [128, k, cols]
        x_all = const.tile([128, KT, BS], mm_dt, tag="x_all")
        w1_all = const.tile([128, KT, RH], mm_dt, tag="w1_all")

        def x_mm(k, c):
            return x_all[:, k, c * CHUNK:(c + 1) * CHUNK]

        def w1_mm(k, ht):
            return w1_all[:, k, ht * 128:(ht + 1) * 128]

        def emit_x_load(c, per_k=False):
            sl = slice(c * CHUNK, (c + 1) * CHUNK)
            if per_k:
                assert c == 0
                for k in range(KT):
                    nc.sync.dma_start(x_all[:, k, sl], x0k[k, :, :])
            else:
                nc.sync.dma_start(x_all[:, :, sl], xw[c, :, :, :])

        def emit_w1_wave(w):
            lo, hi = w * WHT * 128, (w + 1) * WHT * 128
            nc.sync.dma_start(w1_all[:, :, lo:hi], w1w[w, :, :, :])

        gm = ctx.enter_context(tc.tile_pool(name="gm", bufs=2))
        gbcp = ctx.enter_context(tc.tile_pool(name="gbcp", bufs=2))
        srecp = ctx.enter_context(tc.tile_pool(name="srecp", bufs=2))
        hgp = ctx.enter_context(tc.tile_pool(name="hgp", bufs=1))
        tmpp = ctx.enter_context(tc.tile_pool(name="tmpp", bufs=3))
        w2p = ctx.enter_context(tc.tile_pool(name="w2p", bufs=3))
        outp = ctx.enter_context(tc.tile_pool(name="outp", bufs=3))
        p1 = ctx.enter_context(tc.tile_pool(name="p1", bufs=4, space="PSUM"))
        p2 = ctx.enter_context(tc.tile_pool(name="p2", bufs=2, space="PSUM"))
        pbc = ctx.enter_context(tc.tile_pool(name="pbc", bufs=2, space="PSUM"))

        gbcs = {}
        recs = {}
        srecs = {}

        # HAM warm-up: the PE clock sits at 1.2 GHz until ~3.4us of
        # sustained matmul activity.  The first ~12us are DMA-bound idle,
        # so burn them on dummy matmuls over a memset tile -- the real
        # prologue work (gate logits, first h-tiles) then runs at 2.4 GHz.
        warm = const.tile([128, CHUNK], mm_dt, tag="warm")
        nc.any.memset(warm[:], 0.0)

        def emit_ham_warm(n, label):
            for i in range(n):
                pw = pbc.tile([128, CHUNK], f32, tag="pb",
                              name=f"warm_{label}_{i}")
                nc.tensor.matmul(pw[:], warm[:, 0:128], warm[:],
                                 start=True, stop=True)

        def emit_gate_logits(c):
            """gate_bf[:, c] = exp(x@Wg.T + bg) (bf16, unnormalized);
            the 1/sum factor is applied at GEMM2 eviction."""
            sl = slice(c * CHUNK, (c + 1) * CHUNK)
            pg = pbc.tile([R, CHUNK], f32, tag="pb", name=f"pg_{c}")
            for k in range(KT):
                nc.tensor.matmul(pg[:], wg_sb[k][:], x_mm(k, c),
                                 start=(k == 0), stop=(k == KT - 1))
            nc.scalar.activation(gate_bf[:, sl], pg[:], AF.Exp, bias=bg_sb[:])
            gbcs[c] = [None] * R

        def emit_gate_bcast(c, r):
            """E row r -> all 128 partitions via a selector matmul; ACT Relu
            eviction is exact since E > 0.  Used for chunk 0, where the
            ~10us DMA round-trip latency would stall the eviction pipe."""
            sl = slice(c * CHUNK, (c + 1) * CHUNK)
            pb = pbc.tile([128, CHUNK], f32, tag="pb", name=f"pbc{r}_{c}")
            nc.tensor.matmul(pb[:], sel_all[:, r * 128:(r + 1) * 128],
                             gate_bf[:, sl], start=True, stop=True)
            g = gbcp.tile([128, CHUNK], mm_dt, tag=f"gbc{r}",
                          name=f"gbc{r}_{c}")
            nc.scalar.activation(g[:], pb[:], AF.Relu)
            gbcs[c][r] = g

        grows = {}

        def emit_gate_scr_write(c):
            """stage E rows to DRAM for later replicating reads (gpsimd
            stream: idle engine, fires the moment Exp completes)."""
            sl = slice(c * CHUNK, (c + 1) * CHUNK)
            grows[c] = nc.gpsimd.dma_start(gate_scr[:, sl], gate_bf[:, sl])

        def emit_gate_bcast_dma(c, r):
            """E row r -> 128 partitions via replicating DMA from DRAM; free
            of PE cost.  Only for chunks >= 1 (60us+ of lead time)."""
            g = gbcp.tile([128, CHUNK], mm_dt, tag=f"gbc{r}",
                          name=f"gbc{r}_{c}")
            src = bass.AP(gate_scr, r * BS + c * CHUNK, [[0, 128], [1, CHUNK]])
            dma = nc.gpsimd.dma_start(g[:], src)
            tile.add_dep_helper(dma.ins, grows[c].ins,
                                reason="gate bcast read after scr write")
            gbcs[c][r] = g

        def emit_gate_sum(c):
            """S = sum_r E -> 1/S."""
            sl = slice(c * CHUNK, (c + 1) * CHUNK)
            ps = pbc.tile([1, CHUNK], f32, tag="pb", name=f"ps_{c}")
            nc.tensor.matmul(ps[:], ones8b[:], gate_bf[:, sl],
                             start=True, stop=True)
            rec = gm.tile([1, CHUNK], f32, tag="rec", name=f"rec_{c}")
            nc.vector.reciprocal(rec[:], ps[:])
            recs[c] = rec

        def emit_srec_bcast(c):
            """broadcast 1/S to 128 partitions via replicating DMA round-trip
            (an fp32 K=1 matmul would work, but any fp32 matmul in the NEFF
            downclocks the whole warm-state PE 2.4->2.0 GHz).  Triggers ride
            the scalar stream; only 4 of these per kernel, with ~40us of
            lead time before first use."""
            sl = slice(c * CHUNK, (c + 1) * CHUNK)
            w = nc.scalar.dma_start(srec_scr[:, sl], recs[c][:])
            srec = srecp.tile([128, CHUNK], f32, tag="srec",
                              name=f"srec_{c}")
            src = bass.AP(srec_scr, c * CHUNK, [[0, 128], [1, CHUNK]])
            dma = nc.scalar.dma_start(srec[:], src)
            tile.add_dep_helper(dma.ins, w.ins,
                                reason="srec bcast read after scr write")
            srecs[c] = srec

        # --- bulk loads in need-order; queue FIFO preserves it.  wave0
        # leads so GEMM1 can start the moment x0 lands; x0 per-k so the
        # gate logits pipeline with its arrival; x1 rides mid-stream so
        # chunk-1's gate logits never stall the in-order PE.
        emit_x_load(0, per_k=True)
        for w in range(4):
            emit_w1_wave(w)
        emit_x_load(1)
        for w in range(4, NWAVE):
            emit_w1_wave(w)

        emit_ham_warm(18, "a")
        emit_gate_logits(0)
        emit_ham_warm(4, "b")   # bridge the logits -> wave0 idle

        for c in range(NCHUNK):
            sl = slice(c * CHUNK, (c + 1) * CHUNK)
            if c >= 1 and c + 1 < NCHUNK:
                emit_x_load(c + 1)

            hgs = []
            for ht in range(HT):
                # chunk 0's bcasts just-in-time ahead of first use (route r
                # first read at ht=4r; emit it at ht=4r-2)
                if c == 0 and ht % GRP == GRP - 2 and ht // GRP + 1 < R:
                    emit_gate_bcast(0, ht // GRP + 1)
                ps1 = p1.tile([128, CHUNK], f32, tag="ps1")
                for k in range(KT):
                    nc.tensor.matmul(ps1[:], w1_mm(k, ht), x_mm(k, c),
                                     start=(k == 0), stop=(k == KT - 1))
                if c == 0 and ht == 0:
                    # after ht0's MMs so the PE doesn't idle waiting on Exp
                    emit_gate_bcast(0, 0)
                tmp = tmpp.tile([128, CHUNK], f32, tag="tmp",
                                name=f"tmp_{c}_{ht}")
                nc.scalar.activation(tmp[:], ps1[:], AF.Relu,
                                     bias=b1_sb[:, ht:ht + 1])
                hg = hgp.tile([128, CHUNK], mm_dt, tag=f"hg{ht}",
                              name=f"hg{ht}_{c}")
                nc.vector.tensor_mul(hg[:], tmp[:], gbcs[c][ht // GRP][:])
                hgs.append(hg)
                # gate aux scattered through the ht loop: each insertion is
                # one cheap MM + one ACT, with deps long met
                if c == 0:
                    if ht == 9:
                        emit_gate_sum(0)
                    elif ht == 11:
                        emit_srec_bcast(0)
                if c + 1 < NCHUNK:
                    if ht == 8:
                        emit_gate_logits(c + 1)
                        emit_gate_scr_write(c + 1)
                        for r in range(R):
                            emit_gate_bcast_dma(c + 1, r)
                    elif ht == 27:
                        emit_gate_sum(c + 1)
                    elif ht == 29:
                        emit_srec_bcast(c + 1)

            for ot in range(OT):
                w2t = w2p.tile([128, RH], mm_dt, tag="w2t")
                nc.sync.dma_start(w2t[:], w2[ot, :, :])
                ps2 = p2.tile([128, CHUNK], f32, tag="ps2")
                for k2 in range(K2T):
                    nc.tensor.matmul(ps2[:],
                                     w2t[:, k2 * 128:(k2 + 1) * 128],
                                     hgs[k2][:],
                                     start=(k2 == 0), stop=False)
                nc.tensor.matmul(ps2[:], b2_sb[:, ot * 128:(ot + 1) * 128],
                                 gate_bf[:, sl], start=False, stop=True)
                osb = outp.tile([128, CHUNK], f32, tag="osb")
                nc.vector.tensor_mul(osb[:], ps2[:], srecs[c][:])
                nc.sync.dma_start(outT[c, ot, :, :], osb[:])
            del gbcs[c], srecs[c], recs[c]

    nc.compile()
    return nc


def _get_nc(mm_dt_name="bfloat16"):
    if mm_dt_name not in _NC_CACHE:
        _NC_CACHE[mm_dt_name] = _build_nc(mm_dt_name)
    return _NC_CACHE[mm_dt_name]


def _prepare_in_maps(inputs, np_mm_dtype):
    x = np.asarray(inputs["x"], np.float32)
    in_proj_w = np.asarray(inputs["in_proj_w"], np.float32)
    in_proj_b = np.asarray(inputs["in_proj_b"], np.float32)
    out_proj_w = np.asarray(inputs["out_proj_w"], np.float32)
    out_proj_b = np.asarray(inputs["out_proj_b"], np.float32)
    W1 = np.asarray(inputs["W1"], np.float32)
    b1 = np.asarray(inputs["b1"], np.float32)
    W2 = np.asarray(inputs["W2"], np.float32)
    b2 = np.asarray(inputs["b2"], np.float32)
    Wg = np.asarray(inputs["Wg"], np.float32)
    bg = np.asarray(inputs["bg"], np.float32)

    Wv = in_proj_w[2 * D:]
    bv = in_proj_b[2 * D:]
    A = out_proj_w @ Wv                       # [D, D]
    ba = out_proj_w @ bv + out_proj_b         # [D]
    W1r = W1.reshape(RH, D)
    W1f = W1r @ A                             # [RH, D]
    b1f = W1r @ ba + b1.reshape(RH)           # [RH]
    W2cat = W2.transpose(0, 2, 1).reshape(RH, DOUT)

    # w1w[w, p, k, j] = W1f[w*WHT*128 + j, k*128 + p]
    w1w_np = np.ascontiguousarray(
        W1f.reshape(NWAVE, WHT * 128, KT, 128).transpose(0, 3, 2, 1))
    b1v_np = np.ascontiguousarray(b1f.reshape(HT, 128).T)
    w2_np = np.ascontiguousarray(
        W2cat.reshape(K2T, 128, OT, 128).transpose(2, 1, 0, 3)
    ).reshape(OT, 128, RH)
    # [p, k*R+r] = Wg[r, k*128+p]: 128B-contiguous per partition line
    wgt_np = np.ascontiguousarray(Wg.reshape(R, KT, 128).transpose(2, 1, 0)
                                  ).reshape(128, KT * R)
    bg_np = np.ascontiguousarray(bg.reshape(R, 1))

    shared = {
        "w1w": w1w_np.astype(np_mm_dtype),
        "b1v": b1v_np,
        "w2": w2_np.astype(np_mm_dtype),
        "b2d": b2.astype(np_mm_dtype),
        "wgt": wgt_np.astype(np_mm_dtype),
        "bgd": bg_np,
        "seld": np.kron(np.eye(R, dtype=np.float32),
                        np.ones((1, 128), np.float32)).astype(np_mm_dtype),
    }
    in_maps = []
    for cc in range(NCORES):
        xs = x[cc * BS:(cc + 1) * BS]          # [BS, D]
        # xw[c, p, k, j] = x[c*CHUNK + j, k*128 + p]
        xw_np = np.ascontiguousarray(
            xs.reshape(NCHUNK, CHUNK, KT, 128).transpose(0, 3, 2, 1))
        m = dict(shared)
        m["xw"] = xw_np.astype(np_mm_dtype)
        # k-major contiguous copy of chunk 0 for the fast prologue path
        m["x0k"] = np.ascontiguousarray(
            m["xw"][0].transpose(1, 0, 2))
        in_maps.append(m)
    return in_maps


def _run(inputs, trace=False, mm_dt_name="bfloat16"):
    import ml_dtypes
    from concourse.bass_utils import run_bass_kernel_spmd

    np_mm = ml_dtypes.bfloat16 if mm_dt_name == "bfloat16" else np.float32
    nc = _get_nc(mm_dt_name)
    in_maps = _prepare_in_maps(inputs, np_mm)
    res = run_bass_kernel_spmd(nc, in_maps, list(range(NCORES)), trace=trace)
    out = np.empty((B, DOUT), np.float32)
    for cc in range(NCORES):
        o = res.results[cc]["outT"]            # [NCHUNK, OT, 128, CHUNK]
        out[cc * BS:(cc + 1) * BS] = (
            o.transpose(0, 3, 1, 2).reshape(BS, DOUT))
    return out, res


def kernel(**inputs):
    out, _ = _run(inputs, trace=False)
    return out


# revision 36
# speedup vs baseline: 1.0150x; 1.0055x over previous
"""Trainium2 Bass kernel for nn_BrainInspiredRouter.

Math (reference, seq_len==1 attention => attn collapses to the V path):
    attended = x @ (out_proj_w @ Wv).T + (out_proj_w @ bv + out_proj_b)
    h        = relu(attended @ W1[r].T + b1[r])          per route r
    route    = h @ W2[r].T + b2[r]
    gate     = softmax(x @ Wg.T + bg)
    out      = sum_r gate[:, r] * route[:, r, :]

Host-side constant folding (weights only, no activations):
    W1f[r]  = W1[r] @ (out_proj_w @ Wv)      -> h = relu(x @ W1f.T + b1f)
    b1f[r]  = W1[r] @ (out_proj_w@bv + out_proj_b) + b1[r]
    W2cat   = W2.transpose(0,2,1).reshape(R*DH, DOUT)
    out     = (gate*h_flat) @ W2cat + gate @ b2

Device (per core, batch-sharded 8 ways, 2048 rows each; feature-major "T"
layout so both GEMMs chain without transposes):
    gate per chunk: logitsT[8,b] -> E=exp(+bg) -> gate_bf (bf16, unnorm);
      per-route partition-broadcast of E rows and of 1/S via K=1 matmuls
      (ones[1,128].T @ row[1,b]) evicted with ACT Relu (exact for E,1/S>0)
      -- no DRAM round-trip, so the chain has ~1us latency, not ~14us.
    main loop per 512-col batch chunk:
      GEMM1: psum[h,b] = sum_k w1T[k,h-tile] x xT[k,b]   (bf16 MMs)
      evict: ACT relu(+b1f) -> f32 tmp; DVE tmp*gate_bcast -> bf16 Hg
      GEMM2: psum[o,b] = sum_k2 w2[k2,o-tile] x Hg[k2,b] + b2 x gate_bf
      evict: DVE ps2*(1/S) -> f32 -> DMA outT

DMA design: HW queue FIFO order == doorbell order == per-engine program
order; every transfer stripes across all 16 DMA engines; doorbell
instructions cost ~600ns each and block on ring-full (data-paced).  So the
sync stream carries ONLY the bulk loads in need-order (x0 per-k so gate
logits pipeline with arrival, w1 in 8 contiguous 1MB waves with x1 rung
mid-stream), while tiny consts ride the scalar-engine doorbell stream.
"""

import numpy as np

B, D, DOUT, R = 16384, 1024, 1024, 8
DH = D // 2            # 512
RH = R * DH            # 4096
NCORES = 8
BS = B // NCORES       # 2048 rows per core
CHUNK = 512
NCHUNK = BS // CHUNK   # 4
KT = D // 128          # 8 k-tiles over D
HT = RH // 128         # 32 h-tiles
K2T = RH // 128        # 32 k-tiles over RH
OT = DOUT // 128       # 8 out-tiles
GRP = DH // 128        # 4 h-tiles per route
NWAVE = 8              # w1 arrival waves
WHT = HT // NWAVE      # 4 h-tiles per wave

_NC_CACHE = {}


def _build_nc(mm_dt_name="bfloat16"):
    from contextlib import ExitStack

    import concourse.bass as bass
    import concourse.mybir as mybir
    import concourse.tile as tile
    from concourse import bacc

    mm_dt = getattr(mybir.dt, mm_dt_name)
    f32 = mybir.dt.float32
    AF = mybir.ActivationFunctionType

    nc = bacc.Bacc("TRN2", target_bir_lowering=False, debug=False,
                   num_devices=NCORES)

    # host-packed so every bulk load is ONE contiguous-source doorbell
    xw = nc.dram_tensor("xw", [NCHUNK, 128, KT, CHUNK], mm_dt,
                        kind="ExternalInput")
    # chunk 0 again, laid out k-major so each per-k transfer is one
    # contiguous 128KB block (the k-sliced xw view is 1KB-strided and ~3x
    # slower to deliver, starving the gate logits and first h-tiles)
    x0k = nc.dram_tensor("x0k", [KT, 128, CHUNK], mm_dt,
                         kind="ExternalInput")
    w1w = nc.dram_tensor("w1w", [NWAVE, 128, KT, WHT * 128], mm_dt,
                         kind="ExternalInput")
    b1v = nc.dram_tensor("b1v", [128, HT], f32, kind="ExternalInput")
    w2 = nc.dram_tensor("w2", [OT, 128, RH], mm_dt, kind="ExternalInput")
    b2d = nc.dram_tensor("b2d", [R, DOUT], mm_dt, kind="ExternalInput")
    wgt = nc.dram_tensor("wgt", [128, KT * R], mm_dt, kind="ExternalInput")
    bgd = nc.dram_tensor("bgd", [R, 1], f32, kind="ExternalInput")
    seld = nc.dram_tensor("seld", [R, R * 128], mm_dt, kind="ExternalInput")
    srec_scr = nc.dram_tensor("srec_scr", [1, BS], f32)
    gate_scr = nc.dram_tensor("gate_scr", [R, BS], mm_dt)
    outT = nc.dram_tensor("outT", [NCHUNK, OT, 128, CHUNK], f32,
                          kind="ExternalOutput")

    with tile.TileContext(nc) as tc, ExitStack() as ctx:
        const = ctx.enter_context(tc.tile_pool(name="const", bufs=1))

        # tiny consts ride the scalar doorbell stream; sync is bulk-only
        bg_sb = const.tile([R, 1], f32, tag="bg")
        nc.scalar.dma_start(bg_sb[:], bgd[:, :])
        ones8b = const.tile([R, 1], mm_dt, tag="ones8b")
        nc.any.memset(ones8b[:], 1.0)
        # row-selector lhsT tiles: sel_all[:, r*128:(r+1)*128] has row r all
        # ones -> (sel_r.T @ E)[p, n] = E[r, n], a partition-broadcast with a
        # K=8 base-0 contraction (K=1 at base partition r is illegal).
        sel_all = const.tile([R, R * 128], mm_dt, tag="sel_all")
        nc.scalar.dma_start(sel_all[:], seld[:, :])

        wg_all = const.tile([128, KT * R], mm_dt, tag="wg_all")
        nc.scalar.dma_start(wg_all[:], wgt[:, :])
        wg_sb = [wg_all[:, k * R:(k + 1) * R] for k in range(KT)]
        b1_sb = const.tile([128, HT], f32, tag="b1")
        nc.scalar.dma_start(b1_sb[:], b1v[:, :])
        b2_sb = const.tile([R, DOUT], mm_dt, tag="b2")
        nc.scalar.dma_start(b2_sb[:], b2d[:, :])

        gate_bf = const.tile([R, BS], mm_dt, tag="gatebf")  # exp(logits)

        # resident x and w1: one 3D SBUF tile each, [128, k, cols]
        x_all = const.tile([128, KT, BS], mm_dt, tag="x_all")
        w1_all = const.tile([128, KT, RH], mm_dt, tag="w1_all")

        def x_mm(k, c):
            return x_all[:, k, c * CHUNK:(c + 1) * CHUNK]

        def w1_mm(k, ht):
            return w1_all[:, k, ht * 128:(ht + 1) * 128]

        def emit_x_load(c, per_k=False):
            sl = slice(c * CHUNK, (c + 1) * CHUNK)
            if per_k:
                assert c == 0
                for k in range(KT):
                    nc.sync.dma_start(x_all[:, k, sl], x0k[k, :, :])
            else:
                nc.sync.dma_start(x_all[:, :, sl], xw[c, :, :, :])

        def emit_w1_wave(w):
            lo, hi = w * WHT * 128, (w + 1) * WHT * 128
            nc.sync.dma_start(w1_all[:, :, lo:hi], w1w[w, :, :, :])

        gm = ctx.enter_context(tc.tile_pool(name="gm", bufs=2))
        gbcp = ctx.enter_context(tc.tile_pool(name="gbcp", bufs=2))
        srecp = ctx.enter_context(tc.tile_pool(name="srecp", bufs=2))
        hgp = ctx.enter_context(tc.tile_pool(name="hgp", bufs=1))
        tmpp = ctx.enter_context(tc.tile_pool(name="tmpp", bufs=3))
        w2p = ctx.enter_context(tc.tile_pool(name="w2p", bufs=3))
        outp = ctx.enter_context(tc.tile_pool(name="outp", bufs=3))
        p1 = ctx.enter_context(tc.tile_pool(name="p1", bufs=4, space="PSUM"))
        p2 = ctx.enter_context(tc.tile_pool(name="p2", bufs=2, space="PSUM"))
        pbc = ctx.enter_context(tc.tile_pool(name="pbc", bufs=2, space="PSUM"))

        gbcs = {}
        recs = {}
        srecs = {}

        # HAM warm-up: the PE clock sits at 1.2 GHz until ~3.4us of
        # sustained matmul activity.  The first ~12us are DMA-bound idle,
        # so burn them on dummy matmuls over a memset tile -- the real
        # prologue work (gate logits, first h-tiles) then runs at 2.4 GHz.
        warm = const.tile([128, CHUNK], mm_dt, tag="warm")
        nc.any.memset(warm[:], 0.0)

        def emit_ham_warm(n, label):
            for i in range(n):
                pw = pbc.tile([128, CHUNK], f32, tag="pb",
                              name=f"warm_{label}_{i}")
                nc.tensor.matmul(pw[:], warm[:, 0:128], warm[:],
                                 start=True, stop=True)

        def emit_gate_logits(c):
            """gate_bf[:, c] = exp(x@Wg.T + bg) (bf16, unnormalized);
            the 1/sum factor is applied at GEMM2 eviction."""
            sl = slice(c * CHUNK, (c + 1) * CHUNK)
            pg = pbc.tile([R, CHUNK], f32, tag="pb", name=f"pg_{c}")
            for k in range(KT):
                nc.tensor.matmul(pg[:], wg_sb[k][:], x_mm(k, c),
                                 start=(k == 0), stop=(k == KT - 1))
            nc.scalar.activation(gate_bf[:, sl], pg[:], AF.Exp, bias=bg_sb[:])
            gbcs[c] = [None] * R

        def emit_gate_bcast(c, r):
            """E row r -> all 128 partitions via a selector matmul; ACT Relu
            eviction is exact since E > 0.  Used for chunk 0, where the
            ~10us DMA round-trip latency would stall the eviction pipe."""
            sl = slice(c * CHUNK, (c + 1) * CHUNK)
            pb = pbc.tile([128, CHUNK], f32, tag="pb", name=f"pbc{r}_{c}")
            nc.tensor.matmul(pb[:], sel_all[:, r * 128:(r + 1) * 128],
                             gate_bf[:, sl], start=True, stop=True)
            g = gbcp.tile([128, CHUNK], mm_dt, tag=f"gbc{r}",
                          name=f"gbc{r}_{c}")
            nc.scalar.activation(g[:], pb[:], AF.Relu)
            gbcs[c][r] = g

        grows = {}

        def emit_gate_scr_write(c):
            """stage E rows to DRAM for later replicating reads (gpsimd
            stream: idle engine, fires the moment Exp completes)."""
            sl = slice(c * CHUNK, (c + 1) * CHUNK)
            grows[c] = nc.gpsimd.dma_start(gate_scr[:, sl], gate_bf[:, sl])

        def emit_gate_bcast_dma(c, r):
            """E row r -> 128 partitions via replicating DMA from DRAM; free
            of PE cost.  Only for chunks >= 1 (60us+ of lead time)."""
            g = gbcp.tile([128, CHUNK], mm_dt, tag=f"gbc{r}",
                          name=f"gbc{r}_{c}")
            src = bass.AP(gate_scr, r * BS + c * CHUNK, [[0, 128], [1, CHUNK]])
            dma = nc.gpsimd.dma_start(g[:], src)
            tile.add_dep_helper(dma.ins, grows[c].ins,
                                reason="gate bcast read after scr write")
            gbcs[c][r] = g

        def emit_gate_sum(c):
            """S = sum_r E -> 1/S."""
            sl = slice(c * CHUNK, (c + 1) * CHUNK)
            ps = pbc.tile([1, CHUNK], f32, tag="pb", name=f"ps_{c}")
            nc.tensor.matmul(ps[:], ones8b[:], gate_bf[:, sl],
                             start=True, stop=True)
            rec = gm.tile([1, CHUNK], f32, tag="rec", name=f"rec_{c}")
            nc.vector.reciprocal(rec[:], ps[:])
            recs[c] = rec

        def emit_srec_bcast(c):
            """broadcast 1/S to 128 partitions via replicating DMA round-trip
            (an fp32 K=1 matmul would work, but any fp32 matmul in the NEFF
            downclocks the whole warm-state PE 2.4->2.0 GHz).  Triggers ride
            the scalar stream; only 4 of these per kernel, with ~40us of
            lead time before first use."""
            sl = slice(c * CHUNK, (c + 1) * CHUNK)
            w = nc.scalar.dma_start(srec_scr[:, sl], recs[c][:])
            srec = srecp.tile([128, CHUNK], f32, tag="srec",
                              name=f"srec_{c}")
            src = bass.AP(srec_scr, c * CHUNK, [[0, 128], [1, CHUNK]])
            dma = nc.scalar.dma_start(srec[:], src)
            tile.add_dep_helper(dma.ins, w.ins,
                                reason="srec bcast read after scr write")
            srecs[c] = srec

        # --- bulk loads in need-order; queue FIFO preserves it.  Few, big
        # doorbells: each costs ~650ns of in-order sync-engine time, so
        # splitting x0 delays wave0's ring more than pipelining gains.
        # x1 rides mid-stream so chunk-1's gate logits never stall the PE.
        emit_x_load(0)
        for w in range(NWAVE // 2):
            emit_w1_wave(w)
        emit_x_load(1)
        for w in range(NWAVE // 2, NWAVE):
            emit_w1_wave(w)

        emit_ham_warm(14, "a")  # bridges barrier-release to x0 arrival:
        # ~6 cold MMs warm the HAM, the rest run warm and keep it warm, so
        # the gate logits and first h-tiles all execute at 2.4 GHz
        emit_gate_logits(0)
        # stage E and ring ALL of chunk 0's r2..r7 broadcast reads now, so
        # they precede chunk 1's on the gpsimd queue (FIFO): emitting them
        # from ht-loop hooks let chunk 1's 1MB of reads cut in line and
        # stalled chunk 0's evictions at ht24
        emit_gate_scr_write(0)
        for r in range(2, R):
            emit_gate_bcast_dma(0, r)
        emit_ham_warm(2, "b")   # bridge the logits -> wave0 idle

        for c in range(NCHUNK):
            sl = slice(c * CHUNK, (c + 1) * CHUNK)
            if c >= 1 and c + 1 < NCHUNK:
                emit_x_load(c + 1)

            hgs = []
            for ht in range(HT):
                # chunk 0's bcasts just-in-time ahead of first use (route r
                # first read at ht=4r; emit it at ht=4r-2).  r0/r1 need the
                # low-latency PE path; r2+ have >10us of lead so the free
                # DMA round-trip (staged by emit_gate_scr_write) suffices.
                if c == 0 and ht == GRP - 2:
                    emit_gate_bcast(0, 1)
                ps1 = p1.tile([128, CHUNK], f32, tag="ps1")
                for k in range(KT):
                    nc.tensor.matmul(ps1[:], w1_mm(k, ht), x_mm(k, c),
                                     start=(k == 0), stop=(k == KT - 1))
                if c == 0 and ht == 0:
                    # after ht0's MMs so the PE doesn't idle waiting on Exp
                    emit_gate_bcast(0, 0)
                tmp = tmpp.tile([128, CHUNK], f32, tag="tmp",
                                name=f"tmp_{c}_{ht}")
                nc.scalar.activation(tmp[:], ps1[:], AF.Relu,
                                     bias=b1_sb[:, ht:ht + 1])
                hg = hgp.tile([128, CHUNK], mm_dt, tag=f"hg{ht}",
                              name=f"hg{ht}_{c}")
                nc.vector.tensor_mul(hg[:], tmp[:], gbcs[c][ht // GRP][:])
                hgs.append(hg)
                # gate aux scattered through the ht loop: each insertion is
                # one cheap MM + one ACT, with deps long met
                if c == 0:
                    if ht == 9:
                        emit_gate_sum(0)
                    elif ht == 11:
                        emit_srec_bcast(0)
                if c + 1 < NCHUNK:
                    if ht == 8:
                        emit_gate_logits(c + 1)
                        emit_gate_scr_write(c + 1)
                        for r in range(R):
                            emit_gate_bcast_dma(c + 1, r)
                    elif ht == 27:
                        emit_gate_sum(c + 1)
                    elif ht == 29:
                        emit_srec_bcast(c + 1)

            for ot in range(OT):
                w2t = w2p.tile([128, RH], mm_dt, tag="w2t")
                nc.sync.dma_start(w2t[:], w2[ot, :, :])
                ps2 = p2.tile([128, CHUNK], f32, tag="ps2")
                for k2 in range(K2T):
                    nc.tensor.matmul(ps2[:],
                                     w2t[:, k2 * 128:(k2 + 1) * 128],
                                     hgs[k2][:],
                                     start=(k2 == 0), stop=False)
                nc.tensor.matmul(ps2[:], b2_sb[:, ot * 128:(ot + 1) * 128],
                                 gate_bf[:, sl], start=False, stop=True)
                osb = outp.tile([128, CHUNK], f32, tag="osb")
                nc.vector.tensor_mul(osb[:], ps2[:], srecs[c][:])
                nc.sync.dma_start(outT[c, ot, :, :], osb[:])
            del gbcs[c], srecs[c], recs[c]

    nc.compile()
    return nc


def _get_nc(mm_dt_name="bfloat16"):
    if mm_dt_name not in _NC_CACHE:
        _NC_CACHE[mm_dt_name] = _build_nc(mm_dt_name)
    return _NC_CACHE[mm_dt_name]


def _prepare_in_maps(inputs, np_mm_dtype):
    x = np.asarray(inputs["x"], np.float32)
    in_proj_w = np.asarray(inputs["in_proj_w"], np.float32)
    in_proj_b = np.asarray(inputs["in_proj_b"], np.float32)
    out_proj_w = np.asarray(inputs["out_proj_w"], np.float32)
    out_proj_b = np.asarray(inputs["out_proj_b"], np.float32)
    W1 = np.asarray(inputs["W1"], np.float32)
    b1 = np.asarray(inputs["b1"], np.float32)
    W2 = np.asarray(inputs["W2"], np.float32)
    b2 = np.asarray(inputs["b2"], np.float32)
    Wg = np.asarray(inputs["Wg"], np.float32)
    bg = np.asarray(inputs["bg"], np.float32)

    Wv = in_proj_w[2 * D:]
    bv = in_proj_b[2 * D:]
    A = out_proj_w @ Wv                       # [D, D]
    ba = out_proj_w @ bv + out_proj_b         # [D]
    W1r = W1.reshape(RH, D)
    W1f = W1r @ A                             # [RH, D]
    b1f = W1r @ ba + b1.reshape(RH)           # [RH]
    W2cat = W2.transpose(0, 2, 1).reshape(RH, DOUT)

    # w1w[w, p, k, j] = W1f[w*WHT*128 + j, k*128 + p]
    w1w_np = np.ascontiguousarray(
        W1f.reshape(NWAVE, WHT * 128, KT, 128).transpose(0, 3, 2, 1))
    b1v_np = np.ascontiguousarray(b1f.reshape(HT, 128).T)
    w2_np = np.ascontiguousarray(
        W2cat.reshape(K2T, 128, OT, 128).transpose(2, 1, 0, 3)
    ).reshape(OT, 128, RH)
    # [p, k*R+r] = Wg[r, k*128+p]: 128B-contiguous per partition line
    wgt_np = np.ascontiguousarray(Wg.reshape(R, KT, 128).transpose(2, 1, 0)
                                  ).reshape(128, KT * R)
    bg_np = np.ascontiguousarray(bg.reshape(R, 1))

    shared = {
        "w1w": w1w_np.astype(np_mm_dtype),
        "b1v": b1v_np,
        "w2": w2_np.astype(np_mm_dtype),
        "b2d": b2.astype(np_mm_dtype),
        "wgt": wgt_np.astype(np_mm_dtype),
        "bgd": bg_np,
        "seld": np.kron(np.eye(R, dtype=np.float32),
                        np.ones((1, 128), np.float32)).astype(np_mm_dtype),
    }
    in_maps = []
    for cc in range(NCORES):
        xs = x[cc * BS:(cc + 1) * BS]          # [BS, D]
        # xw[c, p, k, j] = x[c*CHUNK + j, k*128 + p]
        xw_np = np.ascontiguousarray(
            xs.reshape(NCHUNK, CHUNK, KT, 128).transpose(0, 3, 2, 1))
        m = dict(shared)
        m["xw"] = xw_np.astype(np_mm_dtype)
        # k-major contiguous copy of chunk 0 for the fast prologue path
        m["x0k"] = np.ascontiguousarray(
            m["xw"][0].transpose(1, 0, 2))
        in_maps.append(m)
    return in_maps


def _run(inputs, trace=False, mm_dt_name="bfloat16"):
    import ml_dtypes
    from concourse.bass_utils import run_bass_kernel_spmd

    np_mm = ml_dtypes.bfloat16 if mm_dt_name == "bfloat16" else np.float32
    nc = _get_nc(mm_dt_name)
    in_maps = _prepare_in_maps(inputs, np_mm)
    res = run_bass_kernel_spmd(nc, in_maps, list(range(NCORES)), trace=trace)
    out = np.empty((B, DOUT), np.float32)
    for cc in range(NCORES):
        o = res.results[cc]["outT"]            # [NCHUNK, OT, 128, CHUNK]
        out[cc * BS:(cc + 1) * BS] = (
            o.transpose(0, 3, 1, 2).reshape(BS, DOUT))
    return out, res


def kernel(**inputs):
    out, _ = _run(inputs, trace=False)
    return out


# revision 37
# speedup vs baseline: 1.0161x; 1.0012x over previous
"""Trainium2 Bass kernel for nn_BrainInspiredRouter.

Math (reference, seq_len==1 attention => attn collapses to the V path):
    attended = x @ (out_proj_w @ Wv).T + (out_proj_w @ bv + out_proj_b)
    h        = relu(attended @ W1[r].T + b1[r])          per route r
    route    = h @ W2[r].T + b2[r]
    gate     = softmax(x @ Wg.T + bg)
    out      = sum_r gate[:, r] * route[:, r, :]

Host-side constant folding (weights only, no activations):
    W1f[r]  = W1[r] @ (out_proj_w @ Wv)      -> h = relu(x @ W1f.T + b1f)
    b1f[r]  = W1[r] @ (out_proj_w@bv + out_proj_b) + b1[r]
    W2cat   = W2.transpose(0,2,1).reshape(R*DH, DOUT)
    out     = (gate*h_flat) @ W2cat + gate @ b2

Device (per core, batch-sharded 8 ways, 2048 rows each; feature-major "T"
layout so both GEMMs chain without transposes):
    gate per chunk: logitsT[8,b] -> E=exp(+bg) -> gate_bf (bf16, unnorm);
      per-route partition-broadcast of E rows and of 1/S via K=1 matmuls
      (ones[1,128].T @ row[1,b]) evicted with ACT Relu (exact for E,1/S>0)
      -- no DRAM round-trip, so the chain has ~1us latency, not ~14us.
    main loop per 512-col batch chunk:
      GEMM1: psum[h,b] = sum_k w1T[k,h-tile] x xT[k,b]   (bf16 MMs)
      evict: ACT relu(+b1f) -> f32 tmp; DVE tmp*gate_bcast -> bf16 Hg
      GEMM2: psum[o,b] = sum_k2 w2[k2,o-tile] x Hg[k2,b] + b2 x gate_bf
      evict: DVE ps2*(1/S) -> f32 -> DMA outT

DMA design: HW queue FIFO order == doorbell order == per-engine program
order; every transfer stripes across all 16 DMA engines; doorbell
instructions cost ~600ns each and block on ring-full (data-paced).  So the
sync stream carries ONLY the bulk loads in need-order (x0 per-k so gate
logits pipeline with arrival, w1 in 8 contiguous 1MB waves with x1 rung
mid-stream), while tiny consts ride the scalar-engine doorbell stream.
"""

import numpy as np

B, D, DOUT, R = 16384, 1024, 1024, 8
DH = D // 2            # 512
RH = R * DH            # 4096
NCORES = 8
BS = B // NCORES       # 2048 rows per core
CHUNK = 512
NCHUNK = BS // CHUNK   # 4
KT = D // 128          # 8 k-tiles over D
HT = RH // 128         # 32 h-tiles
K2T = RH // 128        # 32 k-tiles over RH
OT = DOUT // 128       # 8 out-tiles
GRP = DH // 128        # 4 h-tiles per route
NWAVE = 8              # w1 arrival waves
WHT = HT // NWAVE      # 4 h-tiles per wave

_NC_CACHE = {}


def _build_nc(mm_dt_name="bfloat16"):
    from contextlib import ExitStack

    import concourse.bass as bass
    import concourse.mybir as mybir
    import concourse.tile as tile
    from concourse import bacc

    mm_dt = getattr(mybir.dt, mm_dt_name)
    f32 = mybir.dt.float32
    AF = mybir.ActivationFunctionType

    nc = bacc.Bacc("TRN2", target_bir_lowering=False, debug=False,
                   num_devices=NCORES)

    # host-packed so every bulk load is ONE contiguous-source doorbell
    xw = nc.dram_tensor("xw", [NCHUNK, 128, KT, CHUNK], mm_dt,
                        kind="ExternalInput")
    # chunk 0 again, laid out k-major so each per-k transfer is one
    # contiguous 128KB block (the k-sliced xw view is 1KB-strided and ~3x
    # slower to deliver, starving the gate logits and first h-tiles)
    x0k = nc.dram_tensor("x0k", [KT, 128, CHUNK], mm_dt,
                         kind="ExternalInput")
    w1w = nc.dram_tensor("w1w", [NWAVE, 128, KT, WHT * 128], mm_dt,
                         kind="ExternalInput")
    b1v = nc.dram_tensor("b1v", [128, HT], f32, kind="ExternalInput")
    w2 = nc.dram_tensor("w2", [OT, 128, RH], mm_dt, kind="ExternalInput")
    b2d = nc.dram_tensor("b2d", [128, DOUT], mm_dt, kind="ExternalInput")
    wgt = nc.dram_tensor("wgt", [128, KT * 128], mm_dt, kind="ExternalInput")
    bgd = nc.dram_tensor("bgd", [R, 1], f32, kind="ExternalInput")
    seld = nc.dram_tensor("seld", [R, R * 128], mm_dt, kind="ExternalInput")
    srec_scr = nc.dram_tensor("srec_scr", [1, BS], f32)
    gate_scr = nc.dram_tensor("gate_scr", [R, BS], mm_dt)
    outT = nc.dram_tensor("outT", [NCHUNK, OT, 128, CHUNK], f32,
                          kind="ExternalOutput")

    with tile.TileContext(nc) as tc, ExitStack() as ctx:
        const = ctx.enter_context(tc.tile_pool(name="const", bufs=1))

        # tiny consts ride the scalar doorbell stream; sync is bulk-only
        bg_sb = const.tile([R, 1], f32, tag="bg")
        nc.scalar.dma_start(bg_sb[:], bgd[:, :])
        ones8b = const.tile([R, 1], mm_dt, tag="ones8b")
        nc.any.memset(ones8b[:], 1.0)
        # row-selector lhsT tiles: sel_all[:, r*128:(r+1)*128] has row r all
        # ones -> (sel_r.T @ E)[p, n] = E[r, n], a partition-broadcast with a
        # K=8 base-0 contraction (K=1 at base partition r is illegal).
        sel_all = const.tile([R, R * 128], mm_dt, tag="sel_all")
        nc.scalar.dma_start(sel_all[:], seld[:, :])

        # gate weights padded to 128 stationary columns (cols 8-127 zero)
        # so the logits MMs keep the uniform [128,128] stationary shape --
        # mixed-size stationaries break FWL pipelining (~300ns per switch)
        wg_all = const.tile([128, KT * 128], mm_dt, tag="wg_all")
        nc.scalar.dma_start(wg_all[:], wgt[:, :])
        wg_sb = [wg_all[:, k * 128:(k + 1) * 128] for k in range(KT)]
        b1_sb = const.tile([128, HT], f32, tag="b1")
        nc.scalar.dma_start(b1_sb[:], b1v[:, :])
        # b2 padded to a 128-row stationary (rows 8-127 zero): the b2 MM
        # then matches every other GEMM2 MM's shape -- no pipeline break
        b2_sb = const.tile([128, DOUT], mm_dt, tag="b2")
        nc.scalar.dma_start(b2_sb[:], b2d[:, :])

        gate_bf = const.tile([128, BS], mm_dt, tag="gatebf")  # exp(logits)
        nc.any.memset(gate_bf[:], 0.0)   # rows 8-127 stay zero

        # resident x and w1: one 3D SBUF tile each, [128, k, cols]
        x_all = const.tile([128, KT, BS], mm_dt, tag="x_all")
        w1_all = const.tile([128, KT, RH], mm_dt, tag="w1_all")

        def x_mm(k, c):
            return x_all[:, k, c * CHUNK:(c + 1) * CHUNK]

        def w1_mm(k, ht):
            return w1_all[:, k, ht * 128:(ht + 1) * 128]

        def emit_x_load(c, per_k=False):
            sl = slice(c * CHUNK, (c + 1) * CHUNK)
            if per_k:
                assert c == 0
                for k in range(KT):
                    nc.sync.dma_start(x_all[:, k, sl], x0k[k, :, :])
            else:
                nc.sync.dma_start(x_all[:, :, sl], xw[c, :, :, :])

        def emit_w1_wave(w):
            lo, hi = w * WHT * 128, (w + 1) * WHT * 128
            nc.sync.dma_start(w1_all[:, :, lo:hi], w1w[w, :, :, :])

        gm = ctx.enter_context(tc.tile_pool(name="gm", bufs=2))
        gbcp = ctx.enter_context(tc.tile_pool(name="gbcp", bufs=2))
        srecp = ctx.enter_context(tc.tile_pool(name="srecp", bufs=2))
        hgp = ctx.enter_context(tc.tile_pool(name="hgp", bufs=1))
        tmpp = ctx.enter_context(tc.tile_pool(name="tmpp", bufs=3))
        w2p = ctx.enter_context(tc.tile_pool(name="w2p", bufs=3))
        outp = ctx.enter_context(tc.tile_pool(name="outp", bufs=3))
        p1 = ctx.enter_context(tc.tile_pool(name="p1", bufs=4, space="PSUM"))
        p2 = ctx.enter_context(tc.tile_pool(name="p2", bufs=2, space="PSUM"))
        pbc = ctx.enter_context(tc.tile_pool(name="pbc", bufs=2, space="PSUM"))

        gbcs = {}
        recs = {}
        srecs = {}

        # HAM warm-up: the PE clock sits at 1.2 GHz until ~3.4us of
        # sustained matmul activity.  The first ~12us are DMA-bound idle,
        # so burn them on dummy matmuls over a memset tile -- the real
        # prologue work (gate logits, first h-tiles) then runs at 2.4 GHz.
        warm = const.tile([128, CHUNK], mm_dt, tag="warm")
        nc.any.memset(warm[:], 0.0)

        def emit_ham_warm(n, label):
            for i in range(n):
                pw = pbc.tile([128, CHUNK], f32, tag="pb",
                              name=f"warm_{label}_{i}")
                nc.tensor.matmul(pw[:], warm[:, 0:128], warm[:],
                                 start=True, stop=True)

        def emit_gate_logits(c):
            """gate_bf[:, c] = exp(x@Wg.T + bg) (bf16, unnormalized);
            the 1/sum factor is applied at GEMM2 eviction."""
            sl = slice(c * CHUNK, (c + 1) * CHUNK)
            pg = pbc.tile([128, CHUNK], f32, tag="pb", name=f"pg_{c}")
            for k in range(KT):
                nc.tensor.matmul(pg[:], wg_sb[k][:], x_mm(k, c),
                                 start=(k == 0), stop=(k == KT - 1))
            nc.scalar.activation(gate_bf[0:R, sl], pg[0:R, :], AF.Exp,
                                 bias=bg_sb[:])
            gbcs[c] = [None] * R

        def emit_gate_bcast(c, r):
            """E row r -> all 128 partitions via a selector matmul; ACT Relu
            eviction is exact since E > 0.  Used for chunk 0, where the
            ~10us DMA round-trip latency would stall the eviction pipe."""
            sl = slice(c * CHUNK, (c + 1) * CHUNK)
            pb = pbc.tile([128, CHUNK], f32, tag="pb", name=f"pbc{r}_{c}")
            nc.tensor.matmul(pb[:], sel_all[:, r * 128:(r + 1) * 128],
                             gate_bf[0:R, sl], start=True, stop=True)
            g = gbcp.tile([128, CHUNK], mm_dt, tag=f"gbc{r}",
                          name=f"gbc{r}_{c}")
            nc.scalar.activation(g[:], pb[:], AF.Relu)
            gbcs[c][r] = g

        grows = {}

        def emit_gate_scr_write(c):
            """stage E rows to DRAM for later replicating reads (gpsimd
            stream: idle engine, fires the moment Exp completes)."""
            sl = slice(c * CHUNK, (c + 1) * CHUNK)
            grows[c] = nc.gpsimd.dma_start(gate_scr[:, sl], gate_bf[0:R, sl])

        def emit_gate_bcast_dma(c, r):
            """E row r -> 128 partitions via replicating DMA from DRAM; free
            of PE cost.  Only for chunks >= 1 (60us+ of lead time)."""
            g = gbcp.tile([128, CHUNK], mm_dt, tag=f"gbc{r}",
                          name=f"gbc{r}_{c}")
            src = bass.AP(gate_scr, r * BS + c * CHUNK, [[0, 128], [1, CHUNK]])
            dma = nc.gpsimd.dma_start(g[:], src)
            tile.add_dep_helper(dma.ins, grows[c].ins,
                                reason="gate bcast read after scr write")
            gbcs[c][r] = g

        def emit_gate_sum(c):
            """S = sum_r E -> 1/S."""
            sl = slice(c * CHUNK, (c + 1) * CHUNK)
            ps = pbc.tile([1, CHUNK], f32, tag="pb", name=f"ps_{c}")
            nc.tensor.matmul(ps[:], ones8b[:], gate_bf[0:R, sl],
                             start=True, stop=True)
            rec = gm.tile([1, CHUNK], f32, tag="rec", name=f"rec_{c}")
            nc.vector.reciprocal(rec[:], ps[:])
            recs[c] = rec

        def emit_srec_bcast(c):
            """broadcast 1/S to 128 partitions via replicating DMA round-trip
            (an fp32 K=1 matmul would work, but any fp32 matmul in the NEFF
            downclocks the whole warm-state PE 2.4->2.0 GHz).  Triggers ride
            the scalar stream; only 4 of these per kernel, with ~40us of
            lead time before first use."""
            sl = slice(c * CHUNK, (c + 1) * CHUNK)
            w = nc.scalar.dma_start(srec_scr[:, sl], recs[c][:])
            srec = srecp.tile([128, CHUNK], f32, tag="srec",
                              name=f"srec_{c}")
            src = bass.AP(srec_scr, c * CHUNK, [[0, 128], [1, CHUNK]])
            dma = nc.scalar.dma_start(srec[:], src)
            tile.add_dep_helper(dma.ins, w.ins,
                                reason="srec bcast read after scr write")
            srecs[c] = srec

        # --- bulk loads in need-order; queue FIFO preserves it.  Few, big
        # doorbells: each costs ~650ns of in-order sync-engine time, so
        # splitting x0 delays wave0's ring more than pipelining gains.
        # x1 rides mid-stream so chunk-1's gate logits never stall the PE.
        emit_x_load(0)
        for w in range(NWAVE // 2):
            emit_w1_wave(w)
        emit_x_load(1)
        for w in range(NWAVE // 2, NWAVE):
            emit_w1_wave(w)

        emit_ham_warm(14, "a")  # bridges barrier-release to x0 arrival:
        # ~6 cold MMs warm the HAM, the rest run warm and keep it warm, so
        # the gate logits and first h-tiles all execute at 2.4 GHz
        emit_gate_logits(0)
        # stage E and ring ALL of chunk 0's r2..r7 broadcast reads now, so
        # they precede chunk 1's on the gpsimd queue (FIFO): emitting them
        # from ht-loop hooks let chunk 1's 1MB of reads cut in line and
        # stalled chunk 0's evictions at ht24
        emit_gate_scr_write(0)
        for r in range(2, R):
            emit_gate_bcast_dma(0, r)
        emit_ham_warm(2, "b")   # bridge the logits -> wave0 idle

        for c in range(NCHUNK):
            sl = slice(c * CHUNK, (c + 1) * CHUNK)
            if c >= 1 and c + 1 < NCHUNK:
                emit_x_load(c + 1)

            hgs = []
            for ht in range(HT):
                # chunk 0's bcasts just-in-time ahead of first use (route r
                # first read at ht=4r; emit it at ht=4r-2).  r0/r1 need the
                # low-latency PE path; r2+ have >10us of lead so the free
                # DMA round-trip (staged by emit_gate_scr_write) suffices.
                if c == 0 and ht == GRP - 2:
                    emit_gate_bcast(0, 1)
                ps1 = p1.tile([128, CHUNK], f32, tag="ps1")
                for k in range(KT):
                    nc.tensor.matmul(ps1[:], w1_mm(k, ht), x_mm(k, c),
                                     start=(k == 0), stop=(k == KT - 1))
                if c == 0 and ht == 0:
                    # after ht0's MMs so the PE doesn't idle waiting on Exp
                    emit_gate_bcast(0, 0)
                tmp = tmpp.tile([128, CHUNK], f32, tag="tmp",
                                name=f"tmp_{c}_{ht}")
                nc.scalar.activation(tmp[:], ps1[:], AF.Relu,
                                     bias=b1_sb[:, ht:ht + 1])
                hg = hgp.tile([128, CHUNK], mm_dt, tag=f"hg{ht}",
                              name=f"hg{ht}_{c}")
                nc.vector.tensor_mul(hg[:], tmp[:], gbcs[c][ht // GRP][:])
                hgs.append(hg)
                # gate aux scattered through the ht loop: each insertion is
                # one cheap MM + one ACT, with deps long met
                if c == 0:
                    if ht == 9:
                        emit_gate_sum(0)
                    elif ht == 11:
                        emit_srec_bcast(0)
                if c + 1 < NCHUNK:
                    if ht == 8:
                        emit_gate_logits(c + 1)
                        emit_gate_scr_write(c + 1)
                        for r in range(R):
                            emit_gate_bcast_dma(c + 1, r)
                    elif ht == 27:
                        emit_gate_sum(c + 1)
                    elif ht == 29:
                        emit_srec_bcast(c + 1)

            for ot in range(OT):
                w2t = w2p.tile([128, RH], mm_dt, tag="w2t")
                nc.sync.dma_start(w2t[:], w2[ot, :, :])
                ps2 = p2.tile([128, CHUNK], f32, tag="ps2")
                for k2 in range(K2T):
                    nc.tensor.matmul(ps2[:],
                                     w2t[:, k2 * 128:(k2 + 1) * 128],
                                     hgs[k2][:],
                                     start=(k2 == 0), stop=False)
                nc.tensor.matmul(ps2[:], b2_sb[:, ot * 128:(ot + 1) * 128],
                                 gate_bf[:, sl], start=False, stop=True)
                osb = outp.tile([128, CHUNK], f32, tag="osb")
                nc.vector.tensor_mul(osb[:], ps2[:], srecs[c][:])
                nc.sync.dma_start(outT[c, ot, :, :], osb[:])
            del gbcs[c], srecs[c], recs[c]

    nc.compile()
    return nc


def _get_nc(mm_dt_name="bfloat16"):
    if mm_dt_name not in _NC_CACHE:
        _NC_CACHE[mm_dt_name] = _build_nc(mm_dt_name)
    return _NC_CACHE[mm_dt_name]


def _prepare_in_maps(inputs, np_mm_dtype):
    x = np.asarray(inputs["x"], np.float32)
    in_proj_w = np.asarray(inputs["in_proj_w"], np.float32)
    in_proj_b = np.asarray(inputs["in_proj_b"], np.float32)
    out_proj_w = np.asarray(inputs["out_proj_w"], np.float32)
    out_proj_b = np.asarray(inputs["out_proj_b"], np.float32)
    W1 = np.asarray(inputs["W1"], np.float32)
    b1 = np.asarray(inputs["b1"], np.float32)
    W2 = np.asarray(inputs["W2"], np.float32)
    b2 = np.asarray(inputs["b2"], np.float32)
    Wg = np.asarray(inputs["Wg"], np.float32)
    bg = np.asarray(inputs["bg"], np.float32)

    Wv = in_proj_w[2 * D:]
    bv = in_proj_b[2 * D:]
    A = out_proj_w @ Wv                       # [D, D]
    ba = out_proj_w @ bv + out_proj_b         # [D]
    W1r = W1.reshape(RH, D)
    W1f = W1r @ A                             # [RH, D]
    b1f = W1r @ ba + b1.reshape(RH)           # [RH]
    W2cat = W2.transpose(0, 2, 1).reshape(RH, DOUT)

    # w1w[w, p, k, j] = W1f[w*WHT*128 + j, k*128 + p]
    w1w_np = np.ascontiguousarray(
        W1f.reshape(NWAVE, WHT * 128, KT, 128).transpose(0, 3, 2, 1))
    b1v_np = np.ascontiguousarray(b1f.reshape(HT, 128).T)
    w2_np = np.ascontiguousarray(
        W2cat.reshape(K2T, 128, OT, 128).transpose(2, 1, 0, 3)
    ).reshape(OT, 128, RH)
    # [p, k*128+r] = Wg[r, k*128+p] for r<8, zero-padded to 128 cols
    wgt_np = np.zeros((128, KT, 128), np.float32)
    wgt_np[:, :, :R] = Wg.reshape(R, KT, 128).transpose(2, 1, 0)
    wgt_np = np.ascontiguousarray(wgt_np).reshape(128, KT * 128)
    bg_np = np.ascontiguousarray(bg.reshape(R, 1))

    shared = {
        "w1w": w1w_np.astype(np_mm_dtype),
        "b1v": b1v_np,
        "w2": w2_np.astype(np_mm_dtype),
        "b2d": np.concatenate(
            [b2, np.zeros((128 - R, DOUT), np.float32)]).astype(np_mm_dtype),
        "wgt": wgt_np.astype(np_mm_dtype),
        "bgd": bg_np,
        "seld": np.kron(np.eye(R, dtype=np.float32),
                        np.ones((1, 128), np.float32)).astype(np_mm_dtype),
    }
    in_maps = []
    for cc in range(NCORES):
        xs = x[cc * BS:(cc + 1) * BS]          # [BS, D]
        # xw[c, p, k, j] = x[c*CHUNK + j, k*128 + p]
        xw_np = np.ascontiguousarray(
            xs.reshape(NCHUNK, CHUNK, KT, 128).transpose(0, 3, 2, 1))
        m = dict(shared)
        m["xw"] = xw_np.astype(np_mm_dtype)
        # k-major contiguous copy of chunk 0 for the fast prologue path
        m["x0k"] = np.ascontiguousarray(
            m["xw"][0].transpose(1, 0, 2))
        in_maps.append(m)
    return in_maps


def _run(inputs, trace=False, mm_dt_name="bfloat16"):
    import ml_dtypes
    from concourse.bass_utils import run_bass_kernel_spmd

    np_mm = ml_dtypes.bfloat16 if mm_dt_name == "bfloat16" else np.float32
    nc = _get_nc(mm_dt_name)
    in_maps = _prepare_in_maps(inputs, np_mm)
    res = run_bass_kernel_spmd(nc, in_maps, list(range(NCORES)), trace=trace)
    out = np.empty((B, DOUT), np.float32)
    for cc in range(NCORES):
        o = res.results[cc]["outT"]            # [NCHUNK, OT, 128, CHUNK]
        out[cc * BS:(cc + 1) * BS] = (
            o.transpose(0, 3, 1, 2).reshape(BS, DOUT))
    return out, res


def kernel(**inputs):
    out, _ = _run(inputs, trace=False)
    return out
